# revision 8
# baseline (speedup 1.0000x reference)
"""Trainium2 Bass kernel for nn_BidPrefix (segment_reduce).

Reference semantics, per row r (B=65536 rows, S=512 cols):
    cp[k]    = prod(x[r, 0:k])                  (exclusive prefix product)
    survival = cp[bid]
    rate     = cp[mp] - cp[mp+1], or EPS when mp == 0
returned as (survival [B,1] f32, rate_last [B,1] f32).

Design: masked products -- no cumprod materialisation, no dynamic
gather.  For each needed index k:
    cp[k] = reduce_mult_t( (t >= k) ? 1.0 : x[t] )
          = reduce_mult( max(x, (iota >= k)) )
The blend is ONE fused DVE scalar_tensor_tensor per (row-group, k):
    out = (iota cmp k[p]) max x    (cmp is_ge for k=bid,mp; is_gt for mp+1)
The {0,1} mask makes masked lanes contribute exactly 1.0 (multiplying by
1.0 is exact), so each product reproduces the reference's f32 prefix
product bit-for-bit, and k==0 (empty product == 1) needs no special
case.  All 3*G blends of a supertile land in one [128, 3G, 512] tile and
are reduced by a single 3D reduce_mult -> [128, 3G].

Supertile: [128, G*512] with row r = i*128*G + p*G + g (partition-major,
so each partition's DMA chunk is G*2KB contiguous).

Sharding: pure data parallel over the batch axis, B/8 = 8192 rows per
NeuronCore, same NEFF on all 8 cores (SPMD), outputs concatenated.
"""

import numpy as np

import concourse.bacc as bacc
import concourse.mybir as mybir
from concourse.tile import TileContext
from concourse.bass_utils import run_bass_kernel_spmd

f32 = mybir.dt.float32
i32 = mybir.dt.int32
Alu = mybir.AluOpType

N_CORES = 8
B, S = 65536, 512
ROWS = B // N_CORES          # 8192 rows per core
G = 8                        # 512-wide row-groups per supertile
SUPER = 128 * G              # 1024 rows per supertile
N_SUPER = ROWS // SUPER      # 8 supertiles per core
W = G * S
EPS = 1e-7


def build_bass():
    nc = bacc.Bacc()

    x = nc.dram_tensor("x", [ROWS, S], f32, kind="ExternalInput")
    bid_info = nc.dram_tensor("bid_info", [ROWS, 2], i32, kind="ExternalInput")
    surv_out = nc.dram_tensor("survival", [ROWS, 1], f32, kind="ExternalOutput")
    rate_out = nc.dram_tensor("rate_last", [ROWS, 1], f32, kind="ExternalOutput")

    x_v = x.rearrange("(i p g) s -> i p (g s)", p=128, g=G)
    bi_v = bid_info.rearrange("(i p g) c -> i p (g c)", p=128, g=G)
    so_v = surv_out.rearrange("(i p g) c -> i p (g c)", p=128, g=G)
    ro_v = rate_out.rearrange("(i p g) c -> i p (g c)", p=128, g=G)

    with TileContext(nc) as tc:
        with (
            tc.tile_pool(name="const", bufs=1) as cpool,
            tc.tile_pool(name="big", bufs=2) as bpool,
            tc.tile_pool(name="small", bufs=3) as spool,
        ):
            it512i = cpool.tile([128, 512], i32, tag="it512i")
            nc.gpsimd.iota(it512i[:], pattern=[[1, 512]], base=0,
                           channel_multiplier=0)
            it512 = cpool.tile([128, 512], f32, tag="it512")
            nc.vector.tensor_copy(out=it512[:], in_=it512i[:])

            for i in range(N_SUPER):
                xt = bpool.tile([128, W], f32, tag="xt")
                nc.sync.dma_start(out=xt[:], in_=x_v[i])
                bi = spool.tile([128, 2 * G], i32, tag="bi")
                nc.sync.dma_start(out=bi[:], in_=bi_v[i])

                bif = spool.tile([128, 2 * G], f32, tag="bif")
                nc.vector.tensor_copy(out=bif[:], in_=bi[:])
                bif3 = bif[:].rearrange("p (g c) -> p g c", c=2)
                mpf = bif3[:, :, 0:1]    # [128, G, 1] market price
                bidf = bif3[:, :, 1:2]   # [128, G, 1] bid

                # Tiny reads absorb the HWDGE per-queue semaphores before
                # the TensorScalarPtr-encoded STTs (that ISA encoding has
                # too few sync-wait slots to carry them itself).
                sink = spool.tile([128, 2], f32, tag="sink")
                nc.vector.tensor_copy(out=sink[:, 0:1], in_=xt[:, 0:1])

                # blends: BL[:, g*3+j, :]  j=0: k=bid, 1: k=mp, 2: k=mp+1
                BL = bpool.tile([128, 3 * G, S], f32, tag="BL")
                for g in range(G):
                    xg = xt[:, g * S:(g + 1) * S]
                    specs = [
                        (bidf[:, g, :], Alu.is_ge),
                        (mpf[:, g, :], Alu.is_ge),
                        (mpf[:, g, :], Alu.is_gt),
                    ]
                    for j, (kap, cmp) in enumerate(specs):
                        nc.vector.scalar_tensor_tensor(
                            out=BL[:, g * 3 + j, :], in0=it512[:],
                            scalar=kap, in1=xg, op0=cmp, op1=Alu.max)

                raw = spool.tile([128, 3 * G], f32, tag="raw")
                nc.vector.tensor_reduce(out=raw[:], in_=BL[:],
                                        axis=mybir.AxisListType.X, op=Alu.mult)
                raw3 = raw[:].rearrange("p (g j) -> p g j", j=3)
                svraw = raw3[:, :, 0]
                g1raw = raw3[:, :, 1]
                g2raw = raw3[:, :, 2]

                # rate = (g1-g2)*(1-(mp==0)) + EPS*(mp==0)  -- exact select
                m0m = spool.tile([128, G], f32, tag="m0m")
                nc.vector.tensor_scalar(out=m0m[:], in0=mpf, scalar1=0.0,
                                        scalar2=None, op0=Alu.is_equal)
                onem = spool.tile([128, G], f32, tag="onem")
                nc.vector.tensor_scalar(out=onem[:], in0=m0m[:], scalar1=-1.0,
                                        scalar2=1.0, op0=Alu.mult, op1=Alu.add)
                rate0 = spool.tile([128, G], f32, tag="rate0")
                nc.vector.tensor_sub(out=rate0[:], in0=g1raw, in1=g2raw)
                rate1 = spool.tile([128, G], f32, tag="rate1")
                nc.vector.tensor_mul(out=rate1[:], in0=rate0[:], in1=onem[:])
                rate_t = spool.tile([128, G], f32, tag="rate_t")
                nc.vector.scalar_tensor_tensor(
                    out=rate_t[:], in0=m0m[:], scalar=EPS, in1=rate1[:],
                    op0=Alu.mult, op1=Alu.add)

                nc.sync.dma_start(out=so_v[i], in_=svraw)
                nc.sync.dma_start(out=ro_v[i], in_=rate_t[:])
    nc.finalize()
    return nc


_NC_CACHE = None


def _get_nc():
    global _NC_CACHE
    if _NC_CACHE is None:
        _NC_CACHE = build_bass()
    return _NC_CACHE


def kernel(x, bid_info):
    x = np.ascontiguousarray(np.asarray(x, dtype=np.float32))
    bid_info = np.ascontiguousarray(np.asarray(bid_info, dtype=np.int32))
    assert x.shape == (B, S) and bid_info.shape == (B, 2)

    nc = _get_nc()
    in_maps = [
        {
            "x": x[c * ROWS:(c + 1) * ROWS],
            "bid_info": bid_info[c * ROWS:(c + 1) * ROWS],
        }
        for c in range(N_CORES)
    ]
    res = run_bass_kernel_spmd(nc, in_maps, core_ids=list(range(N_CORES)))
    survival = np.concatenate([r["survival"] for r in res.results], axis=0)
    rate_last = np.concatenate([r["rate_last"] for r in res.results], axis=0)
    return survival, rate_last
